# revision 44
# baseline (speedup 1.0000x reference)
"""Causal self-attention head (B=4, T=4096, C=1024, H=64) on 8 trn2 NeuronCores.

Sharding: each batch is handled by 2 cores. The 32 query blocks (128 rows each)
of a batch are split by parity: core h in {0,1} owns blocks {2p+h}. This makes
the causal work per core identical up to one fully-masked block per slot, so a
single SPMD Bass program serves all 8 cores; the only per-core data differences
are the input rows and a [P,1] bias vector that kills the one extra block.

Device algorithm (per core, all in one Tile program):
  xT (bf16, host layout [P, NCH, T], columns pre-permuted so the core's own
  query blocks occupy positions 0..15 and the partner's blocks 16..31), loaded
  in 512-column panels ordered by use so projections start ~3us in
  -> K^T (128x4096, duplicated on both partition halves), Q^T (128x2048,
     duplicated), V' (32 tiles of 128x65, last col = 1)
  -> attention runs in groups of GRP=4 query slots (512 t columns). Context
     positions are processed as PAIRS (own block p on PE rows 0:63, partner
     block 16+p on rows 64:127) sharing one 2-bank PSUM tile; both positions
     of a pair have the same causal narrowing width, so the QK matmuls, the
     masks, ONE batched exp ACTIVATE, and the AV matmuls all operate on the
     unmasked column suffix only.
     Masks: diagonal tril (own) / parity-kill [P,1] bias (partner) on the
     diagonal block; the masked prefix columns are simply never computed.
     P^T = exp(0.125*S^T + mask) (no row-max pass: |0.125*S| < ~4 here, so
     exp cannot overflow; softmax is shift-invariant).
     O^T (65 x 128) += V'_blk.T @ P^T ; row 64 accumulates the softmax
     denominator via the ones column of V'.
  -> projection phase g+1 is interleaved into attention group g (g>=1) as PE
     filler: the attention rhythm is ACT(exp)-bound, so proj matmuls run in
     the PE bubbles instead of serializing before the group.
  -> epilogue: transpose O^T on PE, divide by the denominator, DMA out.
"""

import numpy as np
import ml_dtypes

B, T, C, H = 4, 4096, 1024, 64
P = 128                      # partitions / block size
NBLK = T // P                # 32 query blocks per batch
NSLOT = NBLK // 2            # 16 query blocks per core
NEG = -30000.0
SCALE = 0.125                # 1/sqrt(64)
GRP = 4                      # query slots processed together (512 t columns)
LAGP = 2                     # software pipeline: AV runs LAGP pairs behind QK

_cache = {}


def _build_program(split=True):
    import concourse.bass as bass
    import concourse.tile as tile
    from concourse import mybir

    f32 = mybir.dt.float32
    bf16 = mybir.dt.bfloat16
    Exp = mybir.ActivationFunctionType.Exp

    nc = bass.Bass()
    # host layouts are partition-major chunked so each load is a single DMA;
    # x is panel-major so each 512-col panel is one contiguous-per-partition
    # transfer (8KB lines) that can land in phase order
    NPAN = T // 512
    # host panel order is use-order pairs: [0,4, 1,5, 2,6, 3,7] so each
    # 2-panel dma_start is one contiguous 16KB-per-partition descriptor set
    xT = nc.declare_dram_parameter(
        "xT", [P, NPAN, C // P, 512], bf16, isOutput=False)
    wkq = nc.declare_dram_parameter("wkq", [P, C // P, 128], bf16, isOutput=False)
    wkv = nc.declare_dram_parameter("wkv", [P, C // P, 128], bf16, isOutput=False)
    wv = nc.declare_dram_parameter("wv", [P, C // P, 64], bf16, isOutput=False)
    # per-core parity bias: NEG for h=0 cores, 0 for h=1 cores
    hbias = nc.declare_dram_parameter("hbias", [P, 1], f32, isOutput=False)
    # output stored [group, p, slot-in-group, h] so each group is one DMA
    out = nc.declare_dram_parameter(
        "out", [NSLOT // GRP, P, GRP, H], f32, isOutput=True)

    trilnp = np.where(
        np.arange(P)[:, None] <= np.arange(P)[None, :], 0.0, NEG
    ).astype(np.float32)
    tril_d = nc.inline_tensor(trilnp, name="tril")
    ident_d = nc.inline_tensor(np.eye(P, dtype=np.float32), name="ident")
    identb_d = nc.inline_tensor(np.eye(P, dtype=ml_dtypes.bfloat16), name="identb")

    NCH = C // P  # 8 contraction chunks

    with tile.TileContext(nc) as tc:
        with (
            tc.tile_pool(name="sing", bufs=1) as sing,
            tc.tile_pool(name="pwork", bufs=4) as pwork,
            tc.tile_pool(name="owork", bufs=3) as owork,
            tc.tile_pool(name="pjp", bufs=2, space="PSUM") as pjp,
            tc.tile_pool(name="spool", bufs=2, space="PSUM") as spool,
            tc.tile_pool(name="opool", bufs=2, space="PSUM") as opool,
        ):
            # ---- resident SBUF tensors ----
            xt_sb = sing.tile([P, NPAN, NCH, 512], bf16)
            wkq_sb = sing.tile([P, NCH, 128], bf16)
            wkv_sb = sing.tile([P, NCH, 128], bf16)
            wv_sb = sing.tile([P, NCH, 64], bf16)
            kt_sb = sing.tile([P, T], bf16)
            qt_sb = sing.tile([P, T // 2], bf16)
            v_sb = sing.tile([P, NBLK, H + 1], bf16)
            tril_sb = sing.tile([P, P], f32)
            hb_sb = sing.tile([P, 1], f32)
            id_sb = sing.tile([P, P], f32)
            idb_sb = sing.tile([P, P], bf16)
            vt_sb = sing.tile([P, T], bf16)

            # x panels on the Sync queue in use order (host layout is already
            # permuted to use order); the first two panels are split into
            # half-panels so phase 0's first matmul chunks gate on 0.5MB.
            # The DMA subsystem services all QUEUED transfers fair-share, so
            # later panels are released only as earlier ones complete: a
            # 1-element dummy dma reading panel k stalls the in-order sync
            # queue (and everything behind it) until panel k has landed.
            tok_sb = sing.tile([1, NPAN], bf16)
            nc.sync.dma_start(out=xt_sb[:, 0, 0:4, :], in_=xT[:, 0, 0:4, :])
            nc.sync.dma_start(out=xt_sb[:, 0, 4:8, :], in_=xT[:, 0, 4:8, :])
            nc.sync.dma_start(out=xt_sb[:, 1, 0:4, :], in_=xT[:, 1, 0:4, :])
            nc.sync.dma_start(out=xt_sb[:, 1, 4:8, :], in_=xT[:, 1, 4:8, :])
            for k in range(2, NPAN):
                gate = k - 2          # keep ~2 panels in flight
                nc.sync.dma_start(out=tok_sb[0:1, k:k + 1],
                                  in_=xt_sb[0:1, gate, 7, 0:1])
                nc.sync.dma_start(out=xt_sb[:, k, :, :], in_=xT[:, k, :, :])
            nc.gpsimd.dma_start(out=wkq_sb[:, :, :], in_=wkq[:, :, :])
            nc.gpsimd.dma_start(out=wkv_sb[:, :, :], in_=wkv[:, :, :])
            nc.gpsimd.dma_start(out=wv_sb[:, :, :], in_=wv[:, :, :])
            nc.gpsimd.dma_start(out=tril_sb, in_=tril_d[:, :])
            nc.gpsimd.dma_start(out=hb_sb, in_=hbias[:, :])
            nc.gpsimd.dma_start(out=id_sb, in_=ident_d[:, :])
            nc.gpsimd.dma_start(out=idb_sb, in_=identb_d[:, :])

            # warm the ACT exp table (~2.7us load) under the DMA wait so the
            # first real exp doesn't pay it
            warm = sing.tile([1, 1], f32)
            nc.vector.memset(warm, 0.0)
            nc.scalar.activation(warm, warm, Exp)

            # one shared scratch for all HAM keep-warm dummy matmuls (a
            # single tile keeps the pjp rotation unpolluted); plain
            # full-contraction matmuls only (concurrent row-tiled dummies
            # to shared PSUM columns race and crash)
            kw_sb = pjp.tile([64, P], f32, tag="pj", name="kw")

            def warmpad(n, src=None):
                s = idb_sb[:, :] if src is None else src
                for i in range(n):
                    nc.tensor.matmul(
                        kw_sb, lhsT=s[:, 64 * (i % 2):64 * (i % 2) + 64],
                        rhs=s[:, 0:P], start=True, stop=True)

            # bridge the PE-idle window before panel 0 lands (wkq is the
            # first gpsimd load, resident ~3us before the first x chunks)
            warmpad(10, wkq_sb[:, 0, :])

            # ---- projection phase g as a list of op CLUMPS (PE filler) ----
            # Each clump is a list of thunks emitted together: a full 8-chunk
            # accumulation chain stays contiguous on the PE queue so it
            # pipelines back-to-back instead of being chopped into isolated
            # matmuls between attention waits.
            def proj_ops(g):
                clumps = []

                def clump(ops):
                    clumps.append(ops)
                ops = []
                sl = slice(g * 512, (g + 1) * 512)
                sg = 4 + g
                sl2 = slice(sg * 512, (sg + 1) * 512)
                # K^T and Q^T for own cols [g*512,(g+1)*512) via [Wk|Wq] pack
                ps = pjp.tile([P, 512], f32, tag="pj", name=f"ps_kq{g}")
                clump([lambda c=c, g=g, ps=ps: nc.tensor.matmul(
                    ps, lhsT=wkq_sb[:, c, :], rhs=xt_sb[:, 2 * g, c, :],
                    start=(c == 0), stop=(c == NCH - 1)) for c in range(NCH)])

                # dup queue: scalar(ACT) is idle during the serial phases
                # 0/1 but saturated by exp inside groups (phases 2/3 run as
                # filler there), so dups move to the gpsimd queue for g>=2
                dupq = nc.scalar if g < 2 else nc.gpsimd

                def kq_post(ps=ps, sl=sl, dupq=dupq):
                    nc.vector.tensor_copy(kt_sb[0:64, sl], ps[0:64, :])
                    nc.vector.tensor_copy(qt_sb[64:128, sl], ps[64:128, :])
                    # SBUF->SBUF DMA duplicates Q^T to rows 0:64 for the
                    # own-half QK rhs (compute engines cannot cross halves).
                    # Own-half K^T needs no dup: only rows 0:64 are read.
                    dupq.dma_start(out=qt_sb[0:64, sl], in_=qt_sb[64:128, sl])
                clump([kq_post])
                # V^T for the own cols, Wv solo: col-tiled as two
                # concurrent 64-out x 256-col matmuls so the 64-wide pass
                # doesn't waste half the PE array. The second half lands on
                # partition rows 64:128 and is transposed from there
                # directly -- no cross-half duplication.
                psv = pjp.tile([P, 256], f32, tag="pj", name=f"ps_v{g}")

                def v_mm(c, g=g, psv=psv):
                    nc.tensor.matmul(
                        psv[0:64, :], lhsT=wv_sb[:, c, 0:64],
                        rhs=xt_sb[:, 2 * g, c, 0:256],
                        start=(c == 0), stop=(c == NCH - 1),
                        tile_position=(0, 0))
                    nc.tensor.matmul(
                        psv[64:128, :], lhsT=wv_sb[:, c, 0:64],
                        rhs=xt_sb[:, 2 * g, c, 256:512],
                        start=(c == 0), stop=(c == NCH - 1),
                        tile_position=(0, 64))
                clump([lambda c=c: v_mm(c) for c in range(NCH)])

                def v_post(psv=psv, g=g):
                    lo2 = g * 512
                    nc.vector.tensor_copy(vt_sb[0:64, lo2:lo2 + 256],
                                          psv[0:64, :])
                    nc.vector.tensor_copy(vt_sb[64:128, lo2 + 256:lo2 + 512],
                                          psv[64:128, :])
                clump([v_post])
                if g == 0:
                    clump([lambda: warmpad(4)])
                # K^T and V^T for partner cols via [Wk|Wv] pack
                ps2 = pjp.tile([P, 512], f32, tag="pj", name=f"ps_kv{g}")
                clump([lambda c=c, g=g, ps2=ps2: nc.tensor.matmul(
                    ps2, lhsT=wkv_sb[:, c, :], rhs=xt_sb[:, 2 * g + 1, c, :],
                    start=(c == 0), stop=(c == NCH - 1)) for c in range(NCH)])

                def kv_post(ps2=ps2, sl2=sl2):
                    # [Wv|Wk] pack order lands V^T on rows 0:64 (exactly
                    # where the V' transposes read) and K^T on rows 64:128
                    # (exactly the partner-half QK lhsT) -- no cross-half
                    # duplication DMAs at all
                    nc.vector.tensor_copy(vt_sb[0:64, sl2], ps2[0:64, :])
                    nc.vector.tensor_copy(kt_sb[64:128, sl2], ps2[64:128, :])
                clump([kv_post])
                # V' tiles (s-part, h) via PE transpose of V^T blocks
                vps = []
                for sb in list(range(4 * g, 4 * g + 4)) + \
                        list(range(16 + 4 * g, 16 + 4 * g + 4)):
                    def vprime(sb=sb):
                        # own blocks in the second half of a panel live on
                        # partition rows 64:128 (col-tiled V chain); partner
                        # blocks and first halves on rows 0:64
                        b0 = 64 if (sb < 16 and sb % 4 >= 2) else 0
                        ptv = pjp.tile([P, 64], bf16, tag="pj", name=f"ptv{sb}")
                        nc.tensor.transpose(
                            ptv, vt_sb[b0:b0 + 64, sb * P:(sb + 1) * P],
                            idb_sb[b0:b0 + 64, b0:b0 + 64])
                        nc.vector.tensor_copy(v_sb[:, sb, 0:H], ptv)
                        nc.vector.memset(v_sb[:, sb, H:H + 1], 1.0)
                    vps.append(vprime)
                clump(vps[0:2])
                clump(vps[2:4])
                clump(vps[4:6])
                clump(vps[6:8])
                if g == 0:
                    clump([lambda: warmpad(4)])
                return clumps

            def attn_group(g, filler=None, keepwarm=False):
                filler = list(filler or [])
                lo = g * GRP
                npair = lo + GRP
                po = opool.tile([H + 1, GRP * P], f32, tag="o", name=f"po{g}")
                pts = {}
                early_epi = (g == NSLOT // GRP - 1)
                ob4e = owork.tile([P, GRP, H], f32, tag="ob",
                                  name=f"ob{g}") if early_epi else None

                def emit_qk(p):
                    w = max(0, p - lo) * P
                    st = spool.tile([P, 2, GRP * P], f32, tag="s",
                                    name=f"st{g}_{p}")
                    nc.tensor.matmul(
                        st[:, 0, w:], lhsT=kt_sb[0:64, p * P:(p + 1) * P],
                        rhs=qt_sb[0:64, lo * P + w:(lo + GRP) * P],
                        start=True, stop=True, tile_position=(0, 0))
                    nc.tensor.matmul(
                        st[:, 1, w:], lhsT=kt_sb[64:128, (16 + p) * P:(17 + p) * P],
                        rhs=qt_sb[64:128, lo * P + w:(lo + GRP) * P],
                        start=True, stop=True, tile_position=(64, 0))
                    if p >= lo:
                        j = p - lo
                        cs = slice(j * P, (j + 1) * P)
                        nc.vector.tensor_add(st[:, 0, cs], st[:, 0, cs], tril_sb)
                        nc.vector.tensor_scalar_add(st[:, 1, cs], st[:, 1, cs],
                                                    hb_sb)
                    pt = pwork.tile([P, 2, GRP * P], bf16, tag="pt",
                                    name=f"pt{g}_{p}")
                    nc.scalar.activation(pt[:, :, w:], st[:, :, w:], Exp,
                                         scale=SCALE)
                    pts[p] = pt

                def emit_av(p):
                    w = max(0, p - lo) * P
                    pt = pts.pop(p)
                    nc.tensor.matmul(
                        po[:, w:], lhsT=v_sb[:, p, :], rhs=pt[:, 0, w:],
                        start=(p == 0), stop=False)
                    nc.tensor.matmul(
                        po[:, w:], lhsT=v_sb[:, 16 + p, :], rhs=pt[:, 1, w:],
                        start=False, stop=(p == npair - 1))
                    # last group: slot jj's po columns are final after pair
                    # lo+jj, so its epilogue overlaps the remaining pairs
                    # instead of trailing the kernel
                    if early_epi and p >= lo:
                        emit_slot_epi(p - lo)

                def emit_slot_epi(jj):
                    cs = slice(jj * P, (jj + 1) * P)
                    ot = owork.tile([H + 1, P], f32, tag="ot", name=f"ot{g}_{jj}")
                    nc.vector.tensor_copy(ot, po[:, cs])
                    ptr = pjp.tile([P, H + 1], f32, tag="pj", name=f"ptr{g}{jj}")
                    nc.tensor.transpose(ptr, ot, id_sb[0:H + 1, 0:H + 1])
                    rc = owork.tile([P, 1], f32, tag="rc", name=f"rc{g}{jj}")
                    nc.vector.reciprocal(rc, ptr[:, H:H + 1])
                    nc.vector.tensor_scalar_mul(ob4e[:, jj, :], ptr[:, 0:H], rc)
                    # store each slot as it completes: slots 0-2 overlap the
                    # remaining pairs, only slot 3's 32KB trails the kernel
                    nc.sync.dma_start(out=out[g, :, jj, :], in_=ob4e[:, jj, :])

                # QK runs one pair AHEAD of the step cadence: exp[p] then
                # queues on ACT while the PE chews the step's filler clump,
                # so the scalar engine (the attention bottleneck) never
                # starves behind the in-order PE queue.
                nq = 0

                def pump_qk():
                    nonlocal nq
                    if nq < npair:
                        emit_qk(nq)
                        nq += 1
                pump_qk()
                for step in range(npair + LAGP):
                    pump_qk()
                    if keepwarm:
                        # ACT-paced group: a few dummy matmuls per step keep
                        # the PE HAM activity window busy so the following
                        # PE-bound group doesn't start at the cold 1.2GHz
                        warmpad(3)
                    if step >= LAGP:
                        emit_av(step - LAGP)
                    # spread the next phase's projection clumps; a clump's
                    # ops are emitted contiguously so its matmuls stream
                    if filler:
                        remaining_steps = (npair + LAGP) - step
                        k = -(-len(filler) // remaining_steps)
                        for _ in range(k):
                            for op in filler.pop(0):
                                op()
                if keepwarm:
                    warmpad(3)
                # epilogue: transpose, normalize, one batched store per group
                if early_epi:
                    return
                ot = owork.tile([H + 1, GRP * P], f32, tag="ot", name=f"ot{g}")
                nc.vector.tensor_copy(ot, po)
                ob4 = owork.tile([P, GRP, H], f32, tag="ob", name=f"ob{g}")
                for jj in range(GRP):
                    ptr = pjp.tile([P, H + 1], f32, tag="pj", name=f"ptr{g}{jj}")
                    nc.tensor.transpose(
                        ptr, ot[:, jj * P:(jj + 1) * P],
                        id_sb[0:H + 1, 0:H + 1])
                    rc = owork.tile([P, 1], f32, tag="rc", name=f"rc{g}{jj}")
                    nc.vector.reciprocal(rc, ptr[:, H:H + 1])
                    nc.vector.tensor_scalar_mul(ob4[:, jj, :], ptr[:, 0:H], rc)
                nc.sync.dma_start(out=out[g, :, :, :], in_=ob4)

            # phase 0 serial (ordered so the kq/v packs start on panel 0
            # alone; the kv pack needs panel 4), then group 0; phase 1
            # serial; phases 2 and 3 ride inside groups 1 and 2 as filler.
            for cl in proj_ops(0):
                for op in cl:
                    op()
            # phase 1 rides inside group 0: its panels land well before the
            # group starts, and the dense filler keeps the PE HAM-warm (a
            # serial phase 1 after the sparse group 0 ran at cold 1.2GHz)
            attn_group(0, filler=proj_ops(1), keepwarm=True)
            attn_group(1, filler=proj_ops(2))
            attn_group(2, filler=proj_ops(3))
            attn_group(3)

    if split:
        _split_matmul_waits(nc, mybir)
    return nc


def _split_matmul_waits(nc, mybir):
    """Walrus's per-instruction ISA structs encode only one sync-wait each.
    For any compute instruction carrying N>1 waits, hoist N-1 of them onto
    single-wait NoOps placed just before it (before the paired Ldweights for
    a Matmult, so the weight load is gated too). Waiting for each semaphore
    sequentially is equivalent to waiting for all (sems are monotone)."""
    split_types = tuple(
        getattr(mybir, t) for t in (
            "InstMatmult", "InstActivation", "InstTensorTensor",
            "InstTensorScalarPtr", "InstTensorCopy", "InstReciprocal",
            "InstMemset", "InstNoOp", "InstStreamTranspose",
            "InstTensorReduce", "InstCopyPredicated", "InstLdweights",
            "InstDMACopy", "InstDrain", "InstTensorScalar",
        ) if hasattr(mybir, t)
    )
    for f in nc.m.functions:
        for bb in f.blocks:
            newlist = []
            changed = False
            for ins in bb.instructions:
                si = ins.sync_info
                if (isinstance(ins, split_types) and si is not None
                        and si.on_wait and len(si.on_wait) >= 2):
                    changed = True
                    extra, keep = list(si.on_wait[:-1]), [si.on_wait[-1]]
                    nops = [
                        mybir.InstNoOp(
                            name=f"{ins.name}-wsplit{k}",
                            ins=[], outs=[],
                            engine=ins.engine,
                            bass_nofuse=True,
                            sync_info=mybir.SyncInfo(on_wait=[w], on_update=[]),
                        )
                        for k, w in enumerate(extra)
                    ]
                    if newlist and isinstance(newlist[-1], mybir.InstLdweights) \
                            and isinstance(ins, mybir.InstMatmult):
                        ld = newlist.pop()
                        newlist.extend(nops + [ld])
                    else:
                        newlist.extend(nops)
                    ins.sync_info = mybir.SyncInfo(
                        on_wait=keep, on_update=list(si.on_update))
                newlist.append(ins)
            if changed:
                bb.instructions = newlist


def _get_program(split=True):
    key = ("nc", split)
    if key not in _cache:
        _cache[key] = _build_program(split)
    return _cache[key]


def _chunked(a):
    """[C, W] row-major -> [P, C//P, W] partition-major chunked layout."""
    W = a.shape[1]
    return np.ascontiguousarray(
        a.reshape(C // P, P, W).transpose(1, 0, 2))


def _panelized(a):
    """[C, T] -> [P, T//512, C//P, 512] partition-major panel layout."""
    return np.ascontiguousarray(
        a.reshape(C // P, P, T // 512, 512).transpose(1, 2, 0, 3))


def _make_in_maps(x, Wk, Wq, Wv):
    bf16 = ml_dtypes.bfloat16
    wkq_np = _chunked(np.concatenate([Wk, Wq], axis=1).astype(bf16))
    wkv_np = _chunked(np.concatenate([Wv, Wk], axis=1).astype(bf16))
    wv_np = _chunked(Wv.astype(bf16))
    in_maps = []
    for core in range(8):
        b, h = core // 2, core % 2
        order = [2 * p + h for p in range(NSLOT)] + \
                [2 * p + (1 - h) for p in range(NSLOT)]
        rows = np.concatenate(
            [np.arange(blk * P, (blk + 1) * P) for blk in order])
        xTc = _panelized(x[b][rows].T.astype(bf16))
        # device panel order is use-order pairs (own g, partner 4+g)
        xTc = np.ascontiguousarray(xTc[:, [0, 4, 1, 5, 2, 6, 3, 7], :, :])
        hb = np.full((P, 1), NEG if h == 0 else 0.0, dtype=np.float32)
        in_maps.append({
            "xT": xTc, "wkq": wkq_np, "wkv": wkv_np, "wv": wv_np, "hbias": hb,
        })
    return in_maps


def kernel(x, Wk, Wq, Wv, _trace=False, _trace_kwargs=None):
    from concourse.bass_utils import run_bass_kernel_spmd

    x = np.asarray(x, dtype=np.float32)
    Wk = np.asarray(Wk, dtype=np.float32)
    Wq = np.asarray(Wq, dtype=np.float32)
    Wv = np.asarray(Wv, dtype=np.float32)

    nc = _get_program()
    in_maps = _make_in_maps(x, Wk, Wq, Wv)
    kw = dict(_trace_kwargs or {})
    res = run_bass_kernel_spmd(nc, in_maps, core_ids=list(range(8)),
                               trace=_trace, **kw)
    _cache["last_result"] = res

    out = np.empty((B, T, H), dtype=np.float32)
    for core in range(8):
        b, h = core // 2, core % 2
        oc = res.results[core]["out"]  # [NSLOT//GRP, P, GRP, H]
        for s in range(NSLOT):
            blk = 2 * s + h
            out[b, blk * P:(blk + 1) * P, :] = oc[s // GRP, :, s % GRP, :]
    return out


# revision 45
# speedup vs baseline: 1.0455x; 1.0455x over previous
"""Causal self-attention head (B=4, T=4096, C=1024, H=64) on 8 trn2 NeuronCores.

Sharding: each batch is handled by 2 cores. The 32 query blocks (128 rows each)
of a batch are split by parity: core h in {0,1} owns blocks {2p+h}. This makes
the causal work per core identical up to one fully-masked block per slot, so a
single SPMD Bass program serves all 8 cores; the only per-core data differences
are the input rows and a [P,1] bias vector that kills the one extra block.

Device algorithm (per core, all in one Tile program):
  xT (bf16, host layout [P, NCH, T], columns pre-permuted so the core's own
  query blocks occupy positions 0..15 and the partner's blocks 16..31), loaded
  in 512-column panels ordered by use so projections start ~3us in
  -> K^T (128x4096, duplicated on both partition halves), Q^T (128x2048,
     duplicated), V' (32 tiles of 128x65, last col = 1)
  -> attention runs in groups of GRP=4 query slots (512 t columns). Context
     positions are processed as PAIRS (own block p on PE rows 0:63, partner
     block 16+p on rows 64:127) sharing one 2-bank PSUM tile; both positions
     of a pair have the same causal narrowing width, so the QK matmuls, the
     masks, ONE batched exp ACTIVATE, and the AV matmuls all operate on the
     unmasked column suffix only.
     Masks: diagonal tril (own) / parity-kill [P,1] bias (partner) on the
     diagonal block; the masked prefix columns are simply never computed.
     P^T = exp(0.125*S^T + mask) (no row-max pass: |0.125*S| < ~4 here, so
     exp cannot overflow; softmax is shift-invariant).
     O^T (65 x 128) += V'_blk.T @ P^T ; row 64 accumulates the softmax
     denominator via the ones column of V'.
  -> projection phase g+1 is interleaved into attention group g (g>=1) as PE
     filler: the attention rhythm is ACT(exp)-bound, so proj matmuls run in
     the PE bubbles instead of serializing before the group.
  -> epilogue: transpose O^T on PE, divide by the denominator, DMA out.
"""

import numpy as np
import ml_dtypes

B, T, C, H = 4, 4096, 1024, 64
P = 128                      # partitions / block size
NBLK = T // P                # 32 query blocks per batch
NSLOT = NBLK // 2            # 16 query blocks per core
NEG = -30000.0
SCALE = 0.125                # 1/sqrt(64)
GRP = 4                      # query slots processed together (512 t columns)
LAGP = 2                     # software pipeline: AV runs LAGP pairs behind QK

_cache = {}


def _build_program(split=True):
    import concourse.bass as bass
    import concourse.tile as tile
    from concourse import mybir

    f32 = mybir.dt.float32
    bf16 = mybir.dt.bfloat16
    Exp = mybir.ActivationFunctionType.Exp

    nc = bass.Bass()
    # host layouts are partition-major chunked so each load is a single DMA;
    # x is panel-major so each 512-col panel is one contiguous-per-partition
    # transfer (8KB lines) that can land in phase order
    NPAN = T // 512
    # host panel order is use-order pairs: [0,4, 1,5, 2,6, 3,7] so each
    # 2-panel dma_start is one contiguous 16KB-per-partition descriptor set
    xT = nc.declare_dram_parameter(
        "xT", [P, NPAN, C // P, 512], bf16, isOutput=False)
    wkq = nc.declare_dram_parameter("wkq", [P, C // P, 128], bf16, isOutput=False)
    wkv = nc.declare_dram_parameter("wkv", [P, C // P, 128], bf16, isOutput=False)
    wv = nc.declare_dram_parameter("wv", [P, C // P, 64], bf16, isOutput=False)
    # per-core parity bias: NEG for h=0 cores, 0 for h=1 cores
    hbias = nc.declare_dram_parameter("hbias", [P, 1], f32, isOutput=False)
    # output stored [group, p, slot-in-group, h] so each group is one DMA
    out = nc.declare_dram_parameter(
        "out", [NSLOT // GRP, P, GRP, H], f32, isOutput=True)

    trilnp = np.where(
        np.arange(P)[:, None] <= np.arange(P)[None, :], 0.0, NEG
    ).astype(np.float32)
    tril_d = nc.inline_tensor(trilnp, name="tril")
    ident_d = nc.inline_tensor(np.eye(P, dtype=np.float32), name="ident")
    identb_d = nc.inline_tensor(np.eye(P, dtype=ml_dtypes.bfloat16), name="identb")

    NCH = C // P  # 8 contraction chunks

    with tile.TileContext(nc) as tc:
        with (
            tc.tile_pool(name="sing", bufs=1) as sing,
            tc.tile_pool(name="pwork", bufs=4) as pwork,
            tc.tile_pool(name="owork", bufs=3) as owork,
            tc.tile_pool(name="pjp", bufs=2, space="PSUM") as pjp,
            tc.tile_pool(name="spool", bufs=2, space="PSUM") as spool,
            tc.tile_pool(name="opool", bufs=2, space="PSUM") as opool,
        ):
            # ---- resident SBUF tensors ----
            xt_sb = sing.tile([P, NPAN, NCH, 512], bf16)
            wkq_sb = sing.tile([P, NCH, 128], bf16)
            wkv_sb = sing.tile([P, NCH, 128], bf16)
            wv_sb = sing.tile([P, NCH, 64], bf16)
            kt_sb = sing.tile([P, T], bf16)
            qt_sb = sing.tile([P, T // 2], bf16)
            v_sb = sing.tile([P, NBLK, H + 1], bf16)
            tril_sb = sing.tile([P, P], f32)
            hb_sb = sing.tile([P, 1], f32)
            id_sb = sing.tile([P, P], f32)
            idb_sb = sing.tile([P, P], bf16)
            vt_sb = sing.tile([P, T], bf16)

            # x panels on the Sync queue in use order (host layout is already
            # permuted to use order); the first two panels are split into
            # half-panels so phase 0's first matmul chunks gate on 0.5MB.
            # The DMA subsystem services all QUEUED transfers fair-share, so
            # later panels are released only as earlier ones complete: a
            # 1-element dummy dma reading panel k stalls the in-order sync
            # queue (and everything behind it) until panel k has landed.
            tok_sb = sing.tile([1, NPAN], bf16)
            nc.sync.dma_start(out=xt_sb[:, 0, 0:4, :], in_=xT[:, 0, 0:4, :])
            nc.sync.dma_start(out=xt_sb[:, 0, 4:8, :], in_=xT[:, 0, 4:8, :])
            nc.sync.dma_start(out=xt_sb[:, 1, 0:4, :], in_=xT[:, 1, 0:4, :])
            nc.sync.dma_start(out=xt_sb[:, 1, 4:8, :], in_=xT[:, 1, 4:8, :])
            for k in range(2, NPAN):
                gate = k - 2          # keep ~2 panels in flight
                nc.sync.dma_start(out=tok_sb[0:1, k:k + 1],
                                  in_=xt_sb[0:1, gate, 7, 0:1])
                nc.sync.dma_start(out=xt_sb[:, k, :, :], in_=xT[:, k, :, :])
            nc.gpsimd.dma_start(out=wkq_sb[:, :, :], in_=wkq[:, :, :])
            nc.gpsimd.dma_start(out=wkv_sb[:, :, :], in_=wkv[:, :, :])
            nc.gpsimd.dma_start(out=wv_sb[:, :, :], in_=wv[:, :, :])
            nc.gpsimd.dma_start(out=tril_sb, in_=tril_d[:, :])
            nc.gpsimd.dma_start(out=hb_sb, in_=hbias[:, :])
            nc.gpsimd.dma_start(out=id_sb, in_=ident_d[:, :])
            nc.gpsimd.dma_start(out=idb_sb, in_=identb_d[:, :])

            # warm the ACT exp table (~2.7us load) under the DMA wait so the
            # first real exp doesn't pay it
            warm = sing.tile([1, 1], f32)
            nc.vector.memset(warm, 0.0)
            nc.scalar.activation(warm, warm, Exp)

            # one shared scratch for all HAM keep-warm dummy matmuls (a
            # single tile keeps the pjp rotation unpolluted); plain
            # full-contraction matmuls only (concurrent row-tiled dummies
            # to shared PSUM columns race and crash)
            kw_sb = pjp.tile([64, P], f32, tag="pj", name="kw")

            def warmpad(n, src=None):
                s = idb_sb[:, :] if src is None else src
                for i in range(n):
                    nc.tensor.matmul(
                        kw_sb, lhsT=s[:, 64 * (i % 2):64 * (i % 2) + 64],
                        rhs=s[:, 0:P], start=True, stop=True)

            # bridge the PE-idle window before panel 0 lands (wkq is the
            # first gpsimd load, resident ~3us before the first x chunks)
            warmpad(10, wkq_sb[:, 0, :])

            # ---- projection phase g as a list of op CLUMPS (PE filler) ----
            # Each clump is a list of thunks emitted together: a full 8-chunk
            # accumulation chain stays contiguous on the PE queue so it
            # pipelines back-to-back instead of being chopped into isolated
            # matmuls between attention waits.
            def proj_ops(g):
                clumps = []

                def clump(ops):
                    clumps.append(ops)
                ops = []
                sl = slice(g * 512, (g + 1) * 512)
                sg = 4 + g
                sl2 = slice(sg * 512, (sg + 1) * 512)
                # K^T and Q^T for own cols [g*512,(g+1)*512) via [Wk|Wq] pack
                ps = pjp.tile([P, 512], f32, tag="pj", name=f"ps_kq{g}")
                clump([lambda c=c, g=g, ps=ps: nc.tensor.matmul(
                    ps, lhsT=wkq_sb[:, c, :], rhs=xt_sb[:, 2 * g, c, :],
                    start=(c == 0), stop=(c == NCH - 1)) for c in range(NCH)])

                # dup queue: scalar(ACT) is idle during the serial phases
                # 0/1 but saturated by exp inside groups (phases 2/3 run as
                # filler there), so dups move to the gpsimd queue for g>=2
                dupq = nc.scalar if g < 2 else nc.gpsimd

                def kq_post(ps=ps, sl=sl, dupq=dupq):
                    nc.vector.tensor_copy(kt_sb[0:64, sl], ps[0:64, :])
                    nc.vector.tensor_copy(qt_sb[64:128, sl], ps[64:128, :])
                    # SBUF->SBUF DMA duplicates Q^T to rows 0:64 for the
                    # own-half QK rhs (compute engines cannot cross halves).
                    # Own-half K^T needs no dup: only rows 0:64 are read.
                    dupq.dma_start(out=qt_sb[0:64, sl], in_=qt_sb[64:128, sl])
                clump([kq_post])
                # V^T for the own cols, Wv solo: col-tiled as two
                # concurrent 64-out x 256-col matmuls so the 64-wide pass
                # doesn't waste half the PE array. The second half lands on
                # partition rows 64:128 and is transposed from there
                # directly -- no cross-half duplication.
                psv = pjp.tile([P, 256], f32, tag="pj", name=f"ps_v{g}")

                def v_mm(c, g=g, psv=psv):
                    nc.tensor.matmul(
                        psv[0:64, :], lhsT=wv_sb[:, c, 0:64],
                        rhs=xt_sb[:, 2 * g, c, 0:256],
                        start=(c == 0), stop=(c == NCH - 1),
                        tile_position=(0, 0))
                    nc.tensor.matmul(
                        psv[64:128, :], lhsT=wv_sb[:, c, 0:64],
                        rhs=xt_sb[:, 2 * g, c, 256:512],
                        start=(c == 0), stop=(c == NCH - 1),
                        tile_position=(0, 64))
                clump([lambda c=c: v_mm(c) for c in range(NCH)])

                def v_post(psv=psv, g=g):
                    lo2 = g * 512
                    nc.vector.tensor_copy(vt_sb[0:64, lo2:lo2 + 256],
                                          psv[0:64, :])
                    nc.vector.tensor_copy(vt_sb[64:128, lo2 + 256:lo2 + 512],
                                          psv[64:128, :])
                clump([v_post])
                if g == 0:
                    clump([lambda: warmpad(4)])
                # K^T and V^T for partner cols via [Wk|Wv] pack
                ps2 = pjp.tile([P, 512], f32, tag="pj", name=f"ps_kv{g}")
                clump([lambda c=c, g=g, ps2=ps2: nc.tensor.matmul(
                    ps2, lhsT=wkv_sb[:, c, :], rhs=xt_sb[:, 2 * g + 1, c, :],
                    start=(c == 0), stop=(c == NCH - 1)) for c in range(NCH)])

                def kv_post(ps2=ps2, sl2=sl2):
                    # [Wv|Wk] pack order lands V^T on rows 0:64 (exactly
                    # where the V' transposes read) and K^T on rows 64:128
                    # (exactly the partner-half QK lhsT) -- no cross-half
                    # duplication DMAs at all
                    nc.vector.tensor_copy(vt_sb[0:64, sl2], ps2[0:64, :])
                    nc.vector.tensor_copy(kt_sb[64:128, sl2], ps2[64:128, :])
                clump([kv_post])
                # V' tiles (s-part, h) via PE transpose of V^T blocks
                vps = []
                for sb in list(range(4 * g, 4 * g + 4)) + \
                        list(range(16 + 4 * g, 16 + 4 * g + 4)):
                    def vprime(sb=sb):
                        # own blocks in the second half of a panel live on
                        # partition rows 64:128 (col-tiled V chain); partner
                        # blocks and first halves on rows 0:64
                        b0 = 64 if (sb < 16 and sb % 4 >= 2) else 0
                        ptv = pjp.tile([P, 64], bf16, tag="pj", name=f"ptv{sb}")
                        nc.tensor.transpose(
                            ptv, vt_sb[b0:b0 + 64, sb * P:(sb + 1) * P],
                            idb_sb[b0:b0 + 64, b0:b0 + 64])
                        nc.vector.tensor_copy(v_sb[:, sb, 0:H], ptv)
                        nc.vector.memset(v_sb[:, sb, H:H + 1], 1.0)
                    vps.append(vprime)
                clump(vps[0:2])
                clump(vps[2:4])
                clump(vps[4:6])
                clump(vps[6:8])
                if g == 0:
                    clump([lambda: warmpad(4)])
                return clumps

            def attn_group(g, filler=None, keepwarm=False):
                filler = list(filler or [])
                lo = g * GRP
                npair = lo + GRP
                po = opool.tile([H + 1, GRP * P], f32, tag="o", name=f"po{g}")
                pts = {}
                early_epi = (g == NSLOT // GRP - 1)
                ob4e = owork.tile([P, GRP, H], f32, tag="ob",
                                  name=f"ob{g}") if early_epi else None

                def emit_qk(p):
                    w = max(0, p - lo) * P
                    st = spool.tile([P, 2, GRP * P], f32, tag="s",
                                    name=f"st{g}_{p}")
                    nc.tensor.matmul(
                        st[:, 0, w:], lhsT=kt_sb[0:64, p * P:(p + 1) * P],
                        rhs=qt_sb[0:64, lo * P + w:(lo + GRP) * P],
                        start=True, stop=True, tile_position=(0, 0))
                    nc.tensor.matmul(
                        st[:, 1, w:], lhsT=kt_sb[64:128, (16 + p) * P:(17 + p) * P],
                        rhs=qt_sb[64:128, lo * P + w:(lo + GRP) * P],
                        start=True, stop=True, tile_position=(64, 0))
                    if p >= lo:
                        j = p - lo
                        cs = slice(j * P, (j + 1) * P)
                        nc.vector.tensor_add(st[:, 0, cs], st[:, 0, cs], tril_sb)
                        nc.vector.tensor_scalar_add(st[:, 1, cs], st[:, 1, cs],
                                                    hb_sb)
                    pt = pwork.tile([P, 2, GRP * P], bf16, tag="pt",
                                    name=f"pt{g}_{p}")
                    nc.scalar.activation(pt[:, :, w:], st[:, :, w:], Exp,
                                         scale=SCALE)
                    pts[p] = pt

                def emit_av(p):
                    w = max(0, p - lo) * P
                    pt = pts.pop(p)
                    nc.tensor.matmul(
                        po[:, w:], lhsT=v_sb[:, p, :], rhs=pt[:, 0, w:],
                        start=(p == 0), stop=False)
                    nc.tensor.matmul(
                        po[:, w:], lhsT=v_sb[:, 16 + p, :], rhs=pt[:, 1, w:],
                        start=False, stop=(p == npair - 1))
                    # last group: slot jj's po columns are final after pair
                    # lo+jj, so its epilogue overlaps the remaining pairs
                    # instead of trailing the kernel
                    if early_epi and p >= lo:
                        emit_slot_epi(p - lo)

                def emit_slot_epi(jj):
                    cs = slice(jj * P, (jj + 1) * P)
                    ot = owork.tile([H + 1, P], f32, tag="ot", name=f"ot{g}_{jj}")
                    nc.vector.tensor_copy(ot, po[:, cs])
                    ptr = pjp.tile([P, H + 1], f32, tag="pj", name=f"ptr{g}{jj}")
                    nc.tensor.transpose(ptr, ot, id_sb[0:H + 1, 0:H + 1])
                    rc = owork.tile([P, 1], f32, tag="rc", name=f"rc{g}{jj}")
                    nc.vector.reciprocal(rc, ptr[:, H:H + 1])
                    nc.vector.tensor_scalar_mul(ob4e[:, jj, :], ptr[:, 0:H], rc)

                # QK runs one pair AHEAD of the step cadence: exp[p] then
                # queues on ACT while the PE chews the step's filler clump,
                # so the scalar engine (the attention bottleneck) never
                # starves behind the in-order PE queue.
                nq = 0

                def pump_qk():
                    nonlocal nq
                    if nq < npair:
                        emit_qk(nq)
                        nq += 1
                pump_qk()
                for step in range(npair + LAGP):
                    pump_qk()
                    if keepwarm:
                        # ACT-paced group: a few dummy matmuls per step keep
                        # the PE HAM activity window busy so the following
                        # PE-bound group doesn't start at the cold 1.2GHz
                        warmpad(3)
                    if step >= LAGP:
                        emit_av(step - LAGP)
                    # spread the next phase's projection clumps; a clump's
                    # ops are emitted contiguously so its matmuls stream
                    if filler:
                        remaining_steps = (npair + LAGP) - step
                        k = -(-len(filler) // remaining_steps)
                        for _ in range(k):
                            for op in filler.pop(0):
                                op()
                if keepwarm:
                    warmpad(3)
                # epilogue: transpose, normalize, one batched store per group
                if early_epi:
                    nc.sync.dma_start(out=out[g, :, :, :], in_=ob4e)
                    return
                ot = owork.tile([H + 1, GRP * P], f32, tag="ot", name=f"ot{g}")
                nc.vector.tensor_copy(ot, po)
                ob4 = owork.tile([P, GRP, H], f32, tag="ob", name=f"ob{g}")
                for jj in range(GRP):
                    ptr = pjp.tile([P, H + 1], f32, tag="pj", name=f"ptr{g}{jj}")
                    nc.tensor.transpose(
                        ptr, ot[:, jj * P:(jj + 1) * P],
                        id_sb[0:H + 1, 0:H + 1])
                    rc = owork.tile([P, 1], f32, tag="rc", name=f"rc{g}{jj}")
                    nc.vector.reciprocal(rc, ptr[:, H:H + 1])
                    nc.vector.tensor_scalar_mul(ob4[:, jj, :], ptr[:, 0:H], rc)
                nc.sync.dma_start(out=out[g, :, :, :], in_=ob4)

            # phase 0 serial (ordered so the kq/v packs start on panel 0
            # alone; the kv pack needs panel 4), then group 0; phase 1
            # serial; phases 2 and 3 ride inside groups 1 and 2 as filler.
            for cl in proj_ops(0):
                for op in cl:
                    op()
            # phase 1 rides inside group 0: its panels land well before the
            # group starts, and the dense filler keeps the PE HAM-warm (a
            # serial phase 1 after the sparse group 0 ran at cold 1.2GHz)
            attn_group(0, filler=proj_ops(1), keepwarm=True)
            attn_group(1, filler=proj_ops(2))
            attn_group(2, filler=proj_ops(3))
            attn_group(3)

    if split:
        _split_matmul_waits(nc, mybir)
    return nc


def _split_matmul_waits(nc, mybir):
    """Walrus's per-instruction ISA structs encode only one sync-wait each.
    For any compute instruction carrying N>1 waits, hoist N-1 of them onto
    single-wait NoOps placed just before it (before the paired Ldweights for
    a Matmult, so the weight load is gated too). Waiting for each semaphore
    sequentially is equivalent to waiting for all (sems are monotone)."""
    split_types = tuple(
        getattr(mybir, t) for t in (
            "InstMatmult", "InstActivation", "InstTensorTensor",
            "InstTensorScalarPtr", "InstTensorCopy", "InstReciprocal",
            "InstMemset", "InstNoOp", "InstStreamTranspose",
            "InstTensorReduce", "InstCopyPredicated", "InstLdweights",
            "InstDMACopy", "InstDrain", "InstTensorScalar",
        ) if hasattr(mybir, t)
    )
    for f in nc.m.functions:
        for bb in f.blocks:
            newlist = []
            changed = False
            for ins in bb.instructions:
                si = ins.sync_info
                if (isinstance(ins, split_types) and si is not None
                        and si.on_wait and len(si.on_wait) >= 2):
                    changed = True
                    extra, keep = list(si.on_wait[:-1]), [si.on_wait[-1]]
                    nops = [
                        mybir.InstNoOp(
                            name=f"{ins.name}-wsplit{k}",
                            ins=[], outs=[],
                            engine=ins.engine,
                            bass_nofuse=True,
                            sync_info=mybir.SyncInfo(on_wait=[w], on_update=[]),
                        )
                        for k, w in enumerate(extra)
                    ]
                    if newlist and isinstance(newlist[-1], mybir.InstLdweights) \
                            and isinstance(ins, mybir.InstMatmult):
                        ld = newlist.pop()
                        newlist.extend(nops + [ld])
                    else:
                        newlist.extend(nops)
                    ins.sync_info = mybir.SyncInfo(
                        on_wait=keep, on_update=list(si.on_update))
                newlist.append(ins)
            if changed:
                bb.instructions = newlist


def _get_program(split=True):
    key = ("nc", split)
    if key not in _cache:
        _cache[key] = _build_program(split)
    return _cache[key]


def _chunked(a):
    """[C, W] row-major -> [P, C//P, W] partition-major chunked layout."""
    W = a.shape[1]
    return np.ascontiguousarray(
        a.reshape(C // P, P, W).transpose(1, 0, 2))


def _panelized(a):
    """[C, T] -> [P, T//512, C//P, 512] partition-major panel layout."""
    return np.ascontiguousarray(
        a.reshape(C // P, P, T // 512, 512).transpose(1, 2, 0, 3))


def _make_in_maps(x, Wk, Wq, Wv):
    bf16 = ml_dtypes.bfloat16
    wkq_np = _chunked(np.concatenate([Wk, Wq], axis=1).astype(bf16))
    wkv_np = _chunked(np.concatenate([Wv, Wk], axis=1).astype(bf16))
    wv_np = _chunked(Wv.astype(bf16))
    in_maps = []
    for core in range(8):
        b, h = core // 2, core % 2
        order = [2 * p + h for p in range(NSLOT)] + \
                [2 * p + (1 - h) for p in range(NSLOT)]
        rows = np.concatenate(
            [np.arange(blk * P, (blk + 1) * P) for blk in order])
        xTc = _panelized(x[b][rows].T.astype(bf16))
        # device panel order is use-order pairs (own g, partner 4+g)
        xTc = np.ascontiguousarray(xTc[:, [0, 4, 1, 5, 2, 6, 3, 7], :, :])
        hb = np.full((P, 1), NEG if h == 0 else 0.0, dtype=np.float32)
        in_maps.append({
            "xT": xTc, "wkq": wkq_np, "wkv": wkv_np, "wv": wv_np, "hbias": hb,
        })
    return in_maps


def kernel(x, Wk, Wq, Wv, _trace=False, _trace_kwargs=None):
    from concourse.bass_utils import run_bass_kernel_spmd

    x = np.asarray(x, dtype=np.float32)
    Wk = np.asarray(Wk, dtype=np.float32)
    Wq = np.asarray(Wq, dtype=np.float32)
    Wv = np.asarray(Wv, dtype=np.float32)

    nc = _get_program()
    in_maps = _make_in_maps(x, Wk, Wq, Wv)
    kw = dict(_trace_kwargs or {})
    res = run_bass_kernel_spmd(nc, in_maps, core_ids=list(range(8)),
                               trace=_trace, **kw)
    _cache["last_result"] = res

    out = np.empty((B, T, H), dtype=np.float32)
    for core in range(8):
        b, h = core // 2, core % 2
        oc = res.results[core]["out"]  # [NSLOT//GRP, P, GRP, H]
        for s in range(NSLOT):
            blk = 2 * s + h
            out[b, blk * P:(blk + 1) * P, :] = oc[s // GRP, :, s % GRP, :]
    return out
